# revision 1
# baseline (speedup 1.0000x reference)
"""GAT+JumpingKnowledge Trainium2 kernel, 8-core SPMD.

Strategy: partition nodes across 8 cores (contiguous ranges, padded to 6272
rows/core). Per GAT layer: each core transforms its own nodes (h @ W), builds
a gather table row [h_t(256)|alpha_src(8)|pad] in bf16, AllGathers the table
to every core's DRAM, then processes its destination-sorted edge list in
128-node windows: dma_gather of source rows, one-hot (edge->node) matrices
built on-device via iota/is_equal, attention coefficients via PE matmuls,
softmax without max-subtraction (exp values are O(1)), and the weighted
scatter-sum as one-hot^T @ (exp * h_src) accumulated in PSUM. Denominator is
factored out per destination node (den = one-hot^T @ exp), so no per-edge
normalization is needed.
"""

import math

import numpy as np
import ml_dtypes

import concourse.bacc as bacc
import concourse.mybir as mybir
import concourse.tile as tile
from concourse.bass_utils import run_bass_kernel_spmd
from concourse.library_config import mlp
from concourse.masks import make_identity

P = 128
BF = ml_dtypes.bfloat16

FULL_CFG = dict(
    N=50000, E=800000, IN=128, HID=256, HEADS=8, NC=64, L=3, CORES=8,
)


def _derive(cfg):
    d = dict(cfg)
    d["SH"] = d["N"] // d["CORES"]                      # real nodes per core
    d["NW"] = math.ceil(d["SH"] / P)                    # windows per core
    d["SHP"] = d["NW"] * P                              # padded nodes per core
    d["HALF"] = (d["CORES"] // 2) * d["SHP"]            # lo/hi table split
    d["C"] = d["HID"] // d["HEADS"]
    d["ROW"] = 384                                      # bf16: 768B, %256
    d["ROWF"] = 128                                     # final layer row: 256B
    d["OUT_D"] = d["HID"] * (d["L"] + 1) + d["NC"]
    assert d["HALF"] < 32768 and d["CORES"] % 2 == 0
    return d


# ---------------------------------------------------------------- host side


def _wrap_idxs(vals, n_tiles):
    """dma_gather int16 index layout: [128, n_tiles*8]; idx i at
    (i%16, i//16) in the first 16 partitions, replicated to 128."""
    n = n_tiles * P
    idx = np.zeros(n, np.int16)
    idx[: len(vals)] = vals.astype(np.int16)
    arr = idx.reshape(n // 16, 16).T
    return np.tile(arr, (8, 1))


def _preprocess(edge_index, cfg):
    """Sort/shard edges; build per-core gather indices + dst columns with a
    shared (compile-time) per-window tile structure."""
    N, CORES, SH, NW, SHP, HALF = (cfg[k] for k in
                                   ("N", "CORES", "SH", "NW", "SHP", "HALF"))
    loops = np.arange(N, dtype=np.int64)
    src = np.concatenate([np.asarray(edge_index[0]), loops])
    dst = np.concatenate([np.asarray(edge_index[1]), loops])
    # padded global source ids
    src_pad = (src // SH) * SHP + (src % SH)

    core_of = dst // SH
    per_core = []
    for k in range(CORES):
        sel = core_of == k
        s, d = src_pad[sel], dst[sel] - k * SH
        order = np.argsort(d, kind="stable")
        s, d = s[order], d[order]
        win = d // P
        wins = []
        for w in range(NW):
            m = win == w
            sw, dw = s[m], d[m] - w * P
            lo = sw < HALF
            wins.append((sw[lo], dw[lo], sw[~lo] - HALF, dw[~lo]))
        per_core.append(wins)

    Ta = [max(1, max(math.ceil(len(per_core[k][w][0]) / P) for k in range(CORES)))
          for w in range(NW)]
    Tb = [max(1, max(math.ceil(len(per_core[k][w][2]) / P) for k in range(CORES)))
          for w in range(NW)]

    idx_lo, idx_hi, dstc = [], [], []
    for k in range(CORES):
        ilo, ihi, dc = [], [], []
        for w in range(NW):
            slo, dlo, shi, dhi = per_core[k][w]
            ilo.append(_wrap_idxs(slo, Ta[w]))
            ihi.append(_wrap_idxs(shi, Tb[w]))
            for vals, nt in ((dlo, Ta[w]), (dhi, Tb[w])):
                dd = np.full(nt * P, -1.0, np.float32)
                dd[: len(vals)] = vals
                dc.append(dd.reshape(nt, P).T)
            del slo, dlo, shi, dhi
        idx_lo.append(np.hstack(ilo))
        idx_hi.append(np.hstack(ihi))
        dstc.append(np.hstack(dc).astype(BF))
    return dict(Ta=Ta, Tb=Tb, idx_lo=idx_lo, idx_hi=idx_hi, dstc=dstc)


# -------------------------------------------------------------- bass program


def _build(meta, cfg, rep=1):
    N, CORES, SH, NW, SHP = (cfg[k] for k in ("N", "CORES", "SH", "NW", "SHP"))
    IN, HID, HEADS, C, NCL, L = (cfg[k] for k in
                                 ("IN", "HID", "HEADS", "C", "NC", "L"))
    ROW, ROWF, OUT_D = cfg["ROW"], cfg["ROWF"], cfg["OUT_D"]
    Ta, Tb = meta["Ta"], meta["Tb"]
    Tw = [a + b for a, b in zip(Ta, Tb)]
    T_MAX = max(Tw)
    SUM_TA, SUM_TB, SUM_T = sum(Ta), sum(Tb), sum(Tw)
    HGRP = CORES // 2

    bf16, f32 = mybir.dt.bfloat16, mybir.dt.float32
    nc = bacc.Bacc("TRN2", target_bir_lowering=False, debug=False,
                   num_devices=CORES)

    # ---- I/O ----
    xT = nc.dram_tensor("xT", [P, NW * IN], bf16, kind="ExternalInput")
    w0 = nc.dram_tensor("w0", [IN, HID], bf16, kind="ExternalInput")
    wc = nc.dram_tensor("wc", [L, 2, P, HID], bf16, kind="ExternalInput")
    wl = nc.dram_tensor("wl", [2, P, NCL], bf16, kind="ExternalInput")
    asb = nc.dram_tensor("asb", [L, P, HID], bf16, kind="ExternalInput")
    adb = nc.dram_tensor("adb", [L, P, HID], bf16, kind="ExternalInput")
    asl = nc.dram_tensor("asl", [P, NCL], bf16, kind="ExternalInput")
    adl = nc.dram_tensor("adl", [P, NCL], bf16, kind="ExternalInput")
    b0b = nc.dram_tensor("b0b", [P, HID], f32, kind="ExternalInput")
    bcb = nc.dram_tensor("bcb", [L, P, HID], f32, kind="ExternalInput")
    blb = nc.dram_tensor("blb", [P, NCL], f32, kind="ExternalInput")
    idx_lo = nc.dram_tensor("idx_lo", [P, SUM_TA * 8], mybir.dt.int16,
                            kind="ExternalInput")
    idx_hi = nc.dram_tensor("idx_hi", [P, SUM_TB * 8], mybir.dt.int16,
                            kind="ExternalInput")
    dstc = nc.dram_tensor("dstc", [P, SUM_T], bf16, kind="ExternalInput")
    out = nc.dram_tensor("out", [SHP, OUT_D], f32, kind="ExternalOutput")

    cc_in = [nc.dram_tensor(f"cc_in{l}", [SHP, ROW], bf16) for l in range(L)]
    cc_out = [nc.dram_tensor(f"cc_out{l}", [CORES, SHP, ROW], bf16,
                             addr_space="Shared") for l in range(L)]
    cc_in.append(nc.dram_tensor(f"cc_in{L}", [SHP, ROWF], bf16))
    cc_out.append(nc.dram_tensor(f"cc_out{L}", [CORES, SHP, ROWF], bf16,
                                 addr_space="Shared"))

    with tile.TileContext(nc) as tc:
        _emit(tc, locals(), meta, cfg, rep)
    nc.compile()
    return nc


def _emit(tc, tens, meta, cfg, rep=1):
    nc = tc.nc
    bf16, f32 = mybir.dt.bfloat16, mybir.dt.float32
    N, CORES, SH, NW, SHP = (cfg[k] for k in ("N", "CORES", "SH", "NW", "SHP"))
    IN, HID, HEADS, C, NCL, L = (cfg[k] for k in
                                 ("IN", "HID", "HEADS", "C", "NC", "L"))
    ROW, ROWF = cfg["ROW"], cfg["ROWF"]
    Ta, Tb = meta["Ta"], meta["Tb"]
    Tw = [a + b for a, b in zip(Ta, Tb)]
    T_MAX = max(Tw)
    HGRP = CORES // 2

    xT, w0, wc, wl = tens["xT"], tens["w0"], tens["wc"], tens["wl"]
    asb, adb, asl, adl = tens["asb"], tens["adb"], tens["asl"], tens["adl"]
    b0b, bcb, blb = tens["b0b"], tens["bcb"], tens["blb"]
    idx_lo_d, idx_hi_d, dstc_d = tens["idx_lo"], tens["idx_hi"], tens["dstc"]
    out_d, cc_in, cc_out = tens["out"], tens["cc_in"], tens["cc_out"]
    SUM_TA, SUM_TB, SUM_T = sum(Ta), sum(Tb), sum(Tw)

    nc.gpsimd.load_library(mlp)

    import contextlib
    ctx = contextlib.ExitStack()
    with ctx:
        const = ctx.enter_context(tc.tile_pool(name="const", bufs=1))
        sb = ctx.enter_context(tc.tile_pool(name="sb", bufs=2))
        ps = ctx.enter_context(tc.tile_pool(name="ps", bufs=2, space="PSUM"))
        ps1 = ctx.enter_context(tc.tile_pool(name="ps1", bufs=1, space="PSUM"))

        # ---------- resident constants ----------
        ident = const.tile([P, P], bf16)
        make_identity(nc, ident[:])
        iota_t = const.tile([P, P], bf16)
        nc.gpsimd.iota(iota_t[:], pattern=[[1, P]], base=0,
                       channel_multiplier=0,
                       allow_small_or_imprecise_dtypes=True)

        xT_t = const.tile([P, NW, IN], bf16)
        nc.sync.dma_start(out=xT_t[:], in_=xT[:].rearrange(
            "p (w i) -> p w i", w=NW))
        w0_t = const.tile([IN, HID], bf16)
        nc.sync.dma_start(out=w0_t[:], in_=w0[:])
        wc_t = const.tile([P, L, 2, HID], bf16)
        nc.sync.dma_start(out=wc_t[:], in_=wc[:].rearrange(
            "l k p h -> p l k h"))
        wl_t = const.tile([P, 2, NCL], bf16)
        nc.sync.dma_start(out=wl_t[:], in_=wl[:].rearrange("k p h -> p k h"))
        asb_t = const.tile([P, L, HID], bf16)
        nc.sync.dma_start(out=asb_t[:], in_=asb[:].rearrange("l p h -> p l h"))
        adb_t = const.tile([P, L, HID], bf16)
        nc.sync.dma_start(out=adb_t[:], in_=adb[:].rearrange("l p h -> p l h"))
        asl_t = const.tile([P, NCL], bf16)
        nc.sync.dma_start(out=asl_t[:], in_=asl[:])
        adl_t = const.tile([P, NCL], bf16)
        nc.sync.dma_start(out=adl_t[:], in_=adl[:])
        b0b_t = const.tile([P, HID], f32)
        nc.sync.dma_start(out=b0b_t[:], in_=b0b[:])
        bcb_t = const.tile([P, L, HID], f32)
        nc.sync.dma_start(out=bcb_t[:], in_=bcb[:].rearrange("l p h -> p l h"))
        blb_t = const.tile([P, NCL], f32)
        nc.sync.dma_start(out=blb_t[:], in_=blb[:])
        idx_lo_t = const.tile([P, SUM_TA * 8], mybir.dt.int16)
        nc.sync.dma_start(out=idx_lo_t[:], in_=idx_lo_d[:])
        idx_hi_t = const.tile([P, SUM_TB * 8], mybir.dt.int16)
        nc.sync.dma_start(out=idx_hi_t[:], in_=idx_hi_d[:])
        dstc_t = const.tile([P, SUM_T], bf16)
        nc.sync.dma_start(out=dstc_t[:], in_=dstc_d[:])

        h_loc = const.tile([P, NW, HID], bf16)       # node-major activations
        ad_loc = const.tile([P, NW, HEADS], bf16)    # alpha_dst per layer

        # offsets of window w inside concatenated idx/dst arrays
        offA = np.concatenate([[0], np.cumsum(Ta)]).astype(int)
        offB = np.concatenate([[0], np.cumsum(Tb)]).astype(int)
        offT = np.concatenate([[0], np.cumsum(Tw)]).astype(int)

        for _rep in range(rep):
            # ---------------- embed: h0 = x @ W0 + b0 ----------------
            for w in range(NW):
                psum_h = ps.tile([P, HID], f32, tag="mm")
                nc.tensor.matmul(psum_h[:], lhsT=xT_t[:, w, :], rhs=w0_t[:],
                                 start=True, stop=True)
                h0f = sb.tile([P, HID], f32, tag="hf")
                nc.vector.tensor_add(out=h0f[:], in0=psum_h[:], in1=b0b_t[:])
                nc.sync.dma_start(out=out_d[w * P:(w + 1) * P, 0:HID], in_=h0f[:])
                nc.scalar.copy(out=h_loc[:, w, :], in_=h0f[:])

            # ---------------- layers ----------------
            for l in range(cfg.get("EMIT_LAYERS", L + 1)):
                final = l == L
                HO = NCL if final else HID          # feature width this layer
                NH = 1 if final else HEADS          # heads
                CH = HO // NH
                RW = ROWF if final else ROW
                w_t = wl_t if final else wc_t[:, l, :, :]
                as_t = asl_t if final else asb_t[:, l, :]
                ad_t = adl_t if final else adb_t[:, l, :]
                bias_t = blb_t if final else bcb_t[:, l, :]
                col0 = HID * (l + 1)
                tab_lo = cc_out[l].ap()[0:HGRP].rearrange("a b c -> (a b) c")
                tab_hi = cc_out[l].ap()[HGRP:CORES].rearrange("a b c -> (a b) c")

                # ---- transform + table build ----
                for w in range(NW):
                    hT = sb.tile([P, 2, P], bf16, tag="hT")
                    for kk in range(2):
                        tp = ps.tile([P, P], bf16, tag="tp")
                        nc.tensor.transpose(out=tp[:], in_=h_loc[:, w, kk * P:(kk + 1) * P],
                                            identity=ident[:])
                        nc.scalar.copy(out=hT[:, kk, :], in_=tp[:])
                    psum_h = ps.tile([P, HID], f32, tag="mm")
                    for kk in range(2):
                        nc.tensor.matmul(psum_h[:, :HO], lhsT=hT[:, kk, :],
                                         rhs=w_t[:, kk, :],
                                         start=(kk == 0), stop=(kk == 1))
                    tbl = sb.tile([P, RW], bf16, tag="tbl")
                    nc.scalar.copy(out=tbl[:, :HO], in_=psum_h[:, :HO])
                    nc.vector.memset(tbl[:, HO + NH:], 0)
                    # alpha_src / alpha_dst
                    tmp = sb.tile([P, HO], bf16, tag="atmp")
                    nc.vector.tensor_tensor(out=tmp[:], in0=tbl[:, :HO],
                                            in1=as_t[:, :HO],
                                            op=mybir.AluOpType.mult)
                    a_f = sb.tile([P, NH], f32, tag="af")
                    nc.vector.reduce_sum(
                        a_f[:], tmp[:].rearrange("p (h c) -> p h c", h=NH),
                        axis=mybir.AxisListType.X)
                    nc.vector.tensor_copy(out=tbl[:, HO:HO + NH], in_=a_f[:])
                    nc.vector.tensor_tensor(out=tmp[:], in0=tbl[:, :HO],
                                            in1=ad_t[:, :HO],
                                            op=mybir.AluOpType.mult)
                    ad_f = sb.tile([P, NH], f32, tag="adf")
                    nc.vector.reduce_sum(
                        ad_f[:], tmp[:].rearrange("p (h c) -> p h c", h=NH),
                        axis=mybir.AxisListType.X)
                    nc.vector.tensor_copy(out=ad_loc[:, w, :NH], in_=ad_f[:])
                    nc.sync.dma_start(out=cc_in[l][w * P:(w + 1) * P, :], in_=tbl[:])

                # ---- table all-gather ----
                if cfg.get("EMIT_CC", True):
                    nc.gpsimd.collective_compute(
                    "AllGather", mybir.AluOpType.bypass,
                    replica_groups=[list(range(CORES))],
                    ins=[cc_in[l].ap().opt()], outs=[cc_out[l].ap().opt()],
                )

                # ---- edge phase ----
                for w in range(cfg.get("EMIT_WINDOWS", NW)):
                    STG = cfg.get("EDGE_STAGE", 8)
                    ta, tb, t_w = Ta[w], Tb[w], Tw[w]
                    buf = sb.tile([P, T_MAX, RW], bf16, tag="buf")
                    nc.gpsimd.dma_gather(buf[:, :ta, :], tab_lo,
                                         idx_lo_t[:, offA[w] * 8:(offA[w] + ta) * 8],
                                         ta * P, ta * P, RW,
                                         single_packet=False)
                    nc.gpsimd.dma_gather(buf[:, ta:t_w, :], tab_hi,
                                         idx_hi_t[:, offB[w] * 8:(offB[w] + tb) * 8],
                                         tb * P, tb * P, RW,
                                         single_packet=False)
                    if STG <= 1:
                        continue
                    oh = sb.tile([P, T_MAX, P], bf16, tag="oh")
                    ohT = sb.tile([P, T_MAX, P], bf16, tag="ohT")
                    e_ps = ps1.tile([P, T_MAX * NH], f32, tag="eps")
                    for t in range(t_w):
                        nc.vector.tensor_tensor(
                            out=oh[:, t, :],
                            in0=dstc_t[:, offT[w] + t:offT[w] + t + 1].to_broadcast([P, P]),
                            in1=iota_t[:], op=mybir.AluOpType.is_equal)
                        if STG <= 2:
                            continue
                        tp = ps.tile([P, P], bf16, tag="tp")
                        nc.tensor.transpose(out=tp[:], in_=oh[:, t, :],
                                            identity=ident[:])
                        nc.scalar.copy(out=ohT[:, t, :], in_=tp[:])
                        if STG <= 3:
                            continue
                        nc.tensor.matmul(e_ps[:, t * NH:(t + 1) * NH],
                                         lhsT=ohT[:, t, :],
                                         rhs=ad_loc[:, w, :NH],
                                         start=True, stop=True)
                    if STG <= 4:
                        continue
                    # e = alpha_s + alpha_d ; leaky ; exp
                    e_sb = sb.tile([P, T_MAX * NH], f32, tag="esb")
                    nc.vector.tensor_tensor(
                        out=e_sb[:, :t_w * NH],
                        in0=buf[:, :t_w, HO:HO + NH], in1=e_ps[:, :t_w * NH],
                        op=mybir.AluOpType.add)
                    e2 = sb.tile([P, T_MAX * NH], f32, tag="e2")
                    nc.vector.tensor_scalar_mul(e2[:, :t_w * NH],
                                                e_sb[:, :t_w * NH], 0.2)
                    nc.vector.tensor_tensor(out=e_sb[:, :t_w * NH],
                                            in0=e_sb[:, :t_w * NH],
                                            in1=e2[:, :t_w * NH],
                                            op=mybir.AluOpType.max)
                    ex = sb.tile([P, T_MAX * NH], bf16, tag="ex")
                    nc.scalar.activation(ex[:, :t_w * NH], e_sb[:, :t_w * NH],
                                         mybir.ActivationFunctionType.Exp)
                    if STG <= 5:
                        continue
                    # vals = h * exp
                    vals = sb.tile([P, T_MAX, HO], bf16, tag="vals")
                    nc.vector.tensor_tensor(
                        out=vals[:, :t_w, :].rearrange("p t (h c) -> p t h c", h=NH),
                        in0=buf[:, :t_w, :HO].rearrange("p t (h c) -> p t h c", h=NH),
                        in1=ex[:, :t_w * NH].rearrange(
                            "p (t h) -> p t h", t=t_w).to_broadcast([P, t_w, NH, CH]),
                        op=mybir.AluOpType.mult)
                    if STG <= 6:
                        continue
                    den_ps = ps1.tile([P, NH], f32, tag="den")
                    o_ps = ps1.tile([P, HO], f32, tag="ops")
                    for t in range(t_w):
                        nc.tensor.matmul(den_ps[:], lhsT=oh[:, t, :],
                                         rhs=ex[:, t * NH:(t + 1) * NH],
                                         start=(t == 0), stop=(t == t_w - 1))
                        nc.tensor.matmul(o_ps[:], lhsT=oh[:, t, :],
                                         rhs=vals[:, t, :],
                                         start=(t == 0), stop=(t == t_w - 1))
                    if STG <= 7:
                        continue
                    den_i = sb.tile([P, NH], f32, tag="deni")
                    nc.vector.tensor_scalar_add(den_i[:], den_ps[:], 1e-16)
                    nc.vector.reciprocal(den_i[:], den_i[:])
                    hf = sb.tile([P, HO], f32, tag="hf2")
                    nc.vector.tensor_tensor(
                        out=hf[:].rearrange("p (h c) -> p h c", h=NH),
                        in0=o_ps[:].rearrange("p (h c) -> p h c", h=NH),
                        in1=den_i[:].to_broadcast([P, NH, CH]),
                        op=mybir.AluOpType.mult)
                    nc.vector.tensor_add(out=hf[:], in0=hf[:], in1=bias_t[:, :HO])
                    if final:
                        nc.sync.dma_start(out=out_d[w * P:(w + 1) * P,
                                                    col0:col0 + HO], in_=hf[:])
                    else:
                        hr = sb.tile([P, HO], f32, tag="hr")
                        nc.scalar.activation(hr[:], hf[:],
                                             mybir.ActivationFunctionType.Relu)
                        nc.sync.dma_start(out=out_d[w * P:(w + 1) * P,
                                                    col0:col0 + HO], in_=hr[:])
                        nc.scalar.copy(out=h_loc[:, w, :], in_=hr[:])


# ------------------------------------------------------------------ driver


def _make_inmaps(inputs, meta, cfg):
    N, CORES, SH, NW, SHP = (cfg[k] for k in ("N", "CORES", "SH", "NW", "SHP"))
    IN, HID, HEADS, NCL, L = (cfg[k] for k in ("IN", "HID", "HEADS", "NC", "L"))

    x = np.asarray(inputs["x"])
    W0 = np.asarray(inputs["W0"]).astype(BF)
    Wc = np.asarray(inputs["Wc"]).reshape(L, 2, P, HID).astype(BF)
    Wl = np.asarray(inputs["Wl"]).reshape(2, P, NCL).astype(BF)
    a_src_c = np.asarray(inputs["a_src_c"]).reshape(L, HID)
    a_dst_c = np.asarray(inputs["a_dst_c"]).reshape(L, HID)
    a_src_l = np.asarray(inputs["a_src_l"]).reshape(NCL)
    a_dst_l = np.asarray(inputs["a_dst_l"]).reshape(NCL)
    b0 = np.asarray(inputs["b0"])
    bc = np.asarray(inputs["bc"])
    bl = np.asarray(inputs["bl"])

    def bcast(v, dt):
        return np.tile(v[None, :], (P, 1)).astype(dt)

    shared = dict(
        w0=W0, wc=Wc, wl=Wl,
        asb=np.stack([bcast(a_src_c[l], BF) for l in range(L)]),
        adb=np.stack([bcast(a_dst_c[l], BF) for l in range(L)]),
        asl=bcast(a_src_l, BF), adl=bcast(a_dst_l, BF),
        b0b=bcast(b0, np.float32),
        bcb=np.stack([bcast(bc[l], np.float32) for l in range(L)]),
        blb=bcast(bl, np.float32),
    )
    maps = []
    for k in range(CORES):
        xl = np.zeros((SHP, IN), np.float32)
        xl[:SH] = x[k * SH:(k + 1) * SH]
        # xT layout: [IN=feat (partition), NW, P=node]
        xTl = np.ascontiguousarray(xl.reshape(NW, P, IN).transpose(2, 0, 1))
        maps.append(dict(shared,
                         xT=xTl.reshape(P, NW * IN).astype(BF),
                         idx_lo=meta["idx_lo"][k], idx_hi=meta["idx_hi"][k],
                         dstc=meta["dstc"][k]))
    return maps


_CACHE = {}


def _prep(inputs, cfg, rep=1):
    ck = ("meta", cfg["N"], cfg["E"])
    if ck not in _CACHE:
        _CACHE[ck] = _preprocess(np.asarray(inputs["edge_index"]), cfg)
    meta = _CACHE[ck]
    bk = ("nc", cfg["N"], cfg["E"], rep)
    if bk not in _CACHE:
        _CACHE[bk] = _build(meta, cfg, rep)
    mk = ("maps", cfg["N"], cfg["E"])
    if mk not in _CACHE:
        _CACHE[mk] = _make_inmaps(inputs, meta, cfg)
    return meta, _CACHE[bk], _CACHE[mk]


def _make_timed_callable(nc, in_maps, n_cores):
    """Cached-jit executor without output donation (kernel writes every
    output element), inputs pre-staged on device; per-call cost is
    dispatch + execute only."""
    import jax
    from jax.sharding import Mesh, PartitionSpec
    from jax.experimental.shard_map import shard_map
    import concourse.mybir as mybir_
    from concourse import bass2jax as b2j

    b2j.install_neuronx_cc_hook()
    partition_name = nc.partition_id_tensor.name if nc.partition_id_tensor else None
    in_names, out_names, out_avals, zero_outs = [], [], [], []
    for alloc in nc.m.functions[0].allocations:
        if not isinstance(alloc, mybir_.MemoryLocationSet):
            continue
        name = alloc.memorylocations[0].name
        if alloc.kind == "ExternalInput":
            if name != partition_name:
                in_names.append(name)
        elif alloc.kind == "ExternalOutput":
            shape = tuple(alloc.tensor_shape)
            dtype = mybir_.dt.np(alloc.dtype)
            out_names.append(name)
            out_avals.append(jax.core.ShapedArray(shape, dtype))
            zero_outs.append(np.zeros(shape, dtype))
    n_params = len(in_names)
    all_in = in_names + out_names + ([partition_name] if partition_name else [])

    def _body(*args):
        operands = list(args)
        if partition_name is not None:
            operands.append(b2j.partition_id_tensor())
        return tuple(b2j._bass_exec_p.bind(
            *operands, out_avals=tuple(out_avals), in_names=tuple(all_in),
            out_names=tuple(out_names), lowering_input_output_aliases=(),
            sim_require_finite=True, sim_require_nnan=True, nc=nc))

    devices = jax.devices()[:n_cores]
    mesh = Mesh(np.asarray(devices), ("core",))
    nin = n_params + len(out_names)
    sharded = jax.jit(shard_map(_body, mesh=mesh,
                                in_specs=(PartitionSpec("core"),) * nin,
                                out_specs=(PartitionSpec("core"),) * len(out_names),
                                check_rep=False), keep_unused=True)
    concat_in = [np.concatenate([np.asarray(in_maps[c][nm]) for c in range(n_cores)],
                                axis=0) for nm in in_names]
    concat_zeros = [np.zeros((n_cores * z.shape[0], *z.shape[1:]), z.dtype)
                    for z in zero_outs]
    sharding = jax.sharding.NamedSharding(mesh, PartitionSpec("core"))
    dev_args = [jax.device_put(a, sharding) for a in concat_in + concat_zeros]

    def call():
        outs = sharded(*dev_args)
        jax.block_until_ready(outs)
        return outs
    return call


def timed_run(inputs, reps=3, trials=6):
    import time as _t
    cfg = _derive(FULL_CFG)
    _, nc1, in_maps = _prep(inputs, cfg, rep=1)
    _, ncR, _ = _prep(inputs, cfg, rep=reps)
    f1 = _make_timed_callable(nc1, in_maps, cfg["CORES"])
    fR = _make_timed_callable(ncR, in_maps, cfg["CORES"])
    f1(); fR()  # warm-up/compile
    t1s, tRs = [], []
    for _ in range(trials):
        t0 = _t.time(); f1(); t1s.append(_t.time() - t0)
        t0 = _t.time(); fR(); tRs.append(_t.time() - t0)
    t1, tR = min(t1s), min(tRs)
    print(f"[timing] rep1 {t1*1e3:.2f} ms  rep{reps} {tR*1e3:.2f} ms "
          f"(medians {np.median(t1s)*1e3:.2f}/{np.median(tRs)*1e3:.2f})")
    return (tR - t1) / (reps - 1) * 1e9


def _run(inputs, cfg, sim_check=False):
    meta, nc, in_maps = _prep(inputs, cfg)
    N, SH, SHP = cfg["N"], cfg["SH"], cfg["SHP"]
    if sim_check:
        from concourse.bass_interp import MultiCoreSim
        sim = MultiCoreSim(nc, num_cores=cfg["CORES"], require_finite=False,
                           require_nnan=False)
        for k, core in sim.cores.items():
            for name, arr in in_maps[k].items():
                core.tensor(name)[:] = arr
        sim.simulate(check_with_hw=False)
        outs = [np.array(sim.cores[k].tensor("out")) for k in range(cfg["CORES"])]
    else:
        res = run_bass_kernel_spmd(nc, in_maps,
                                   core_ids=list(range(cfg["CORES"])))
        outs = [res.results[k]["out"] for k in range(cfg["CORES"])]
    return np.concatenate([o[:SH] for o in outs], axis=0)


def kernel(**inputs) -> np.ndarray:
    cfg = _derive(FULL_CFG)
    return _run(inputs, cfg)


# note on xT: built as x_local [SHP, IN] -> windows [NW, P, IN] -> transpose
# to [IN, NW, P] so xT_t[:, w, :] is [feat(partition), node(free)] = lhsT.



# revision 2
# speedup vs baseline: 1.4735x; 1.4735x over previous
"""GAT+JumpingKnowledge Trainium2 kernel, 8-core SPMD, v2.

Node-partitioned across 8 cores. Per GAT layer each core transforms its own
nodes (h @ W) into a gather-table row [h | alpha_src | pad] (bf16, 768B), the
rows are AllGathered chunk-by-chunk (window-aligned chunks, overlapped with
the previous layer's edge phase), and each core processes its destination-
sorted edge list in 2-window groups: one dma_gather per table half, both
one-hot matrices (edge-major `oh` and node-major `ohT`) built by single DVE
is_equal ops (ohT from a host-precomputed partition-replicated dst array), a
per-tile PE matmul pair (alpha_dst gather via ohT, weighted scatter-sum via
oh with the softmax denominator fused in as extra columns), and a group-wide
e-pipeline (add / leaky-relu / exp written back into the gather buffer's
alpha_src slot, vals multiply in place). The next layer's transform is fused
into each window's finalize so the table AllGather chunks stream out while
the edge phase is still running.
"""

import math

import numpy as np
import ml_dtypes

import concourse.bacc as bacc
import concourse.mybir as mybir
import concourse.tile as tile
from concourse.bass_utils import run_bass_kernel_spmd
from concourse.library_config import mlp
from concourse.masks import make_identity

P = 128
BF = ml_dtypes.bfloat16

FULL_CFG = dict(
    N=50000, E=800000, IN=128, HID=256, HEADS=8, NC=64, L=3, CORES=8,
    GRP=2,                     # windows per gather group
    LO_CH=(13, 12),            # window counts of lo-table AG chunks
    HI_CH=(12, 12),            # window counts of hi-table AG chunks
)


def _derive(cfg):
    d = dict(cfg)
    d["SH"] = d["N"] // d["CORES"]
    d["NW"] = math.ceil(d["SH"] / P)
    d["SHP"] = d["NW"] * P
    d["C"] = d["HID"] // d["HEADS"]
    d["ROW"] = 384                     # bf16 cols: 768B rows (h 256 | as 8 | pad)
    d["ROWF"] = 128                    # final layer: 256B rows (h 64 | as 1 | pad)
    d["OUT_D"] = d["HID"] * (d["L"] + 1) + d["NC"]
    ch = list(cfg["LO_CH"]) + list(cfg["HI_CH"])
    assert sum(ch) == d["NW"]
    d["CHUNKS"] = ch
    d["CH_W0"] = np.concatenate([[0], np.cumsum(ch)]).astype(int)   # first window
    d["N_LO_W"] = sum(cfg["LO_CH"])
    d["LO_ROWS"] = d["N_LO_W"] * P * d["CORES"]
    d["HI_ROWS"] = (d["NW"] - d["N_LO_W"]) * P * d["CORES"]
    assert d["LO_ROWS"] < 32768 and d["HI_ROWS"] < 32768
    # row offset of chunk c in the flat table
    d["CH_BASE"] = np.concatenate([[0], np.cumsum([c * P * d["CORES"] for c in ch])]).astype(int)
    # groups of windows for gathers
    g = cfg["GRP"]
    d["GROUPS"] = [tuple(range(a, min(a + g, d["NW"]))) for a in range(0, d["NW"], g)]
    return d


def _chunk_of_window(d, w):
    for c in range(len(d["CHUNKS"])):
        if d["CH_W0"][c] <= w < d["CH_W0"][c + 1]:
            return c
    raise AssertionError(w)


# ---------------------------------------------------------------- host side


def _wrap_idxs(vals, n_tiles):
    """dma_gather int16 index layout: [128, n_tiles*8]; idx i at
    (i%16, i//16) in the first 16 partitions, replicated to 128."""
    n = n_tiles * P
    idx = np.zeros(n, np.int16)
    idx[: len(vals)] = vals.astype(np.int16)
    arr = idx.reshape(n // 16, 16).T
    return np.tile(arr, (8, 1))


def _preprocess(edge_index, cfg):
    N, CORES, SH, NW, SHP = (cfg[k] for k in ("N", "CORES", "SH", "NW", "SHP"))
    LO_ROWS, CH_W0, CH_BASE, CHUNKS = (cfg[k] for k in
                                       ("LO_ROWS", "CH_W0", "CH_BASE", "CHUNKS"))
    loops = np.arange(N, dtype=np.int64)
    src = np.concatenate([np.asarray(edge_index[0]), loops])
    dst = np.concatenate([np.asarray(edge_index[1]), loops])

    # chunk-major flat-table row id for every source node
    k_src = src // SH
    r = src % SH
    w_src = r // P
    p_src = r % P
    # chunk id per window
    c_of_w = np.zeros(NW, np.int64)
    for c, nwin in enumerate(CHUNKS):
        c_of_w[CH_W0[c]:CH_W0[c + 1]] = c
    c_src = c_of_w[w_src]
    row_id = (CH_BASE[c_src] + k_src * (np.asarray(CHUNKS)[c_src] * P)
              + (w_src - CH_W0[c_src]) * P + p_src)

    core_of = dst // SH
    per_core = []
    for k in range(CORES):
        sel = core_of == k
        s, dl = row_id[sel], dst[sel] - k * SH
        win = dl // P
        dw = dl % P
        wins = []
        for w in range(NW):
            m = win == w
            sw, dww = s[m], dw[m]
            lo = sw < LO_ROWS
            slo, dlo = sw[lo], dww[lo]
            shi, dhi = sw[~lo] - LO_ROWS, dww[~lo]
            o1 = np.argsort(slo, kind="stable")
            o2 = np.argsort(shi, kind="stable")
            wins.append((slo[o1], dlo[o1], shi[o2], dhi[o2]))
        per_core.append(wins)

    Ta = [max(1, max(math.ceil(len(per_core[k][w][0]) / P) for k in range(CORES)))
          for w in range(NW)]
    Tb = [max(1, max(math.ceil(len(per_core[k][w][2]) / P) for k in range(CORES)))
          for w in range(NW)]

    # group tile structure: per group, tiles in order
    # [lo(w0).. lo(w1).., hi(w0).., hi(w1)..]; per-window tile index lists.
    groups = cfg["GROUPS"]
    g_ta = [sum(Ta[w] for w in g) for g in groups]
    g_tb = [sum(Tb[w] for w in g) for g in groups]
    g_t = [a + b for a, b in zip(g_ta, g_tb)]
    win_tiles = {}   # w -> (list of tile idx within group, group idx)
    for gi, g in enumerate(groups):
        off_lo = 0
        off_hi = g_ta[gi]
        for w in g:
            tl = list(range(off_lo, off_lo + Ta[w]))
            th = list(range(off_hi, off_hi + Tb[w]))
            win_tiles[w] = (tl + th, gi)
            off_lo += Ta[w]
            off_hi += Tb[w]

    idx_lo, idx_hi, dstc, dstT = [], [], [], []
    for k in range(CORES):
        ilo, ihi = [], []
        dc = np.full((sum(g_t), P), -1, np.int8)     # [tile, edge] -> dst-in-window
        toff = 0
        for gi, g in enumerate(groups):
            lo_cols, hi_cols = [], []
            for w in g:
                slo, dlo, shi, dhi = per_core[k][w]
                ilo.append(_wrap_idxs(slo, Ta[w]))
                ihi.append(_wrap_idxs(shi, Tb[w]))
                dd = np.full(Ta[w] * P, -1, np.int8)
                dd[: len(dlo)] = dlo
                lo_cols.append(dd.reshape(Ta[w], P))
                dd = np.full(Tb[w] * P, -1, np.int8)
                dd[: len(dhi)] = dhi
                hi_cols.append(dd.reshape(Tb[w], P))
            blk = np.vstack(lo_cols + hi_cols)       # [g_t, P]
            dc[toff:toff + g_t[gi]] = blk
            toff += g_t[gi]
        idx_lo.append(np.hstack(ilo))
        idx_hi.append(np.hstack(ihi))
        dstc.append(np.ascontiguousarray(dc.T))                    # [P, SUM_T]
        dstT.append(np.tile(dc.reshape(1, -1), (P, 1)))            # [P, SUM_T*P]
    return dict(Ta=Ta, Tb=Tb, g_ta=g_ta, g_tb=g_tb, g_t=g_t,
                win_tiles=win_tiles, idx_lo=idx_lo, idx_hi=idx_hi,
                dstc=dstc, dstT=dstT)


# -------------------------------------------------------------- bass program


def _build(meta, cfg, rep=1):
    CORES, NW, SHP = cfg["CORES"], cfg["NW"], cfg["SHP"]
    IN, HID, NCL, L = cfg["IN"], cfg["HID"], cfg["NC"], cfg["L"]
    ROW, ROWF, OUT_D = cfg["ROW"], cfg["ROWF"], cfg["OUT_D"]
    SUM_TA = sum(meta["Ta"])
    SUM_TB = sum(meta["Tb"])
    SUM_T = SUM_TA + SUM_TB

    bf16, f32 = mybir.dt.bfloat16, mybir.dt.float32
    i8, i16 = mybir.dt.int8, mybir.dt.int16
    nc = bacc.Bacc("TRN2", target_bir_lowering=False, debug=False,
                   num_devices=CORES)

    t = {}
    t["xT"] = nc.dram_tensor("xT", [P, NW * IN], bf16, kind="ExternalInput")
    t["w0"] = nc.dram_tensor("w0", [IN, HID], bf16, kind="ExternalInput")
    t["w0c"] = nc.dram_tensor("w0c", [IN, HID], bf16, kind="ExternalInput")
    t["wc"] = nc.dram_tensor("wc", [L - 1, 2, P, HID], bf16, kind="ExternalInput")
    t["wl"] = nc.dram_tensor("wl", [2, P, NCL], bf16, kind="ExternalInput")
    t["aab"] = nc.dram_tensor("aab", [L, 2, P, HID], bf16, kind="ExternalInput")
    t["aal"] = nc.dram_tensor("aal", [2, P, NCL], bf16, kind="ExternalInput")
    t["b0b"] = nc.dram_tensor("b0b", [P, HID], f32, kind="ExternalInput")
    t["b0cb"] = nc.dram_tensor("b0cb", [P, HID], f32, kind="ExternalInput")
    t["bcb"] = nc.dram_tensor("bcb", [L, P, HID], f32, kind="ExternalInput")
    t["blb"] = nc.dram_tensor("blb", [P, NCL], f32, kind="ExternalInput")
    t["idx_lo"] = nc.dram_tensor("idx_lo", [P, SUM_TA * 8], i16, kind="ExternalInput")
    t["idx_hi"] = nc.dram_tensor("idx_hi", [P, SUM_TB * 8], i16, kind="ExternalInput")
    t["dstc"] = nc.dram_tensor("dstc", [P, SUM_T], i8, kind="ExternalInput")
    t["dstT"] = nc.dram_tensor("dstT", [P, SUM_T * P], i8, kind="ExternalInput")
    t["out"] = nc.dram_tensor("out", [SHP, OUT_D], bf16, kind="ExternalOutput")

    TOT = SHP * CORES
    t["cc_in"] = [nc.dram_tensor(f"cc_in{l}", [SHP, ROW if l < L else ROWF], bf16)
                  for l in range(L + 1)]
    t["tab"] = [nc.dram_tensor(f"tab{l}", [TOT, ROW if l < L else ROWF], bf16,
                               addr_space="Shared") for l in range(L + 1)]

    with tile.TileContext(nc) as tc:
        _emit(tc, t, meta, cfg, rep)
    nc.compile()
    return nc


def _emit(tc, t, meta, cfg, rep=1):
    nc = tc.nc
    bf16, f32 = mybir.dt.bfloat16, mybir.dt.float32
    i8 = mybir.dt.int8
    CORES, NW, SHP = cfg["CORES"], cfg["NW"], cfg["SHP"]
    IN, HID, HEADS, NCL, L = (cfg[k] for k in ("IN", "HID", "HEADS", "NC", "L"))
    ROW, ROWF = cfg["ROW"], cfg["ROWF"]
    LO_ROWS, CH_BASE, CHUNKS, CH_W0 = (cfg[k] for k in
                                       ("LO_ROWS", "CH_BASE", "CHUNKS", "CH_W0"))
    GROUPS = cfg["GROUPS"]
    Ta, Tb = meta["Ta"], meta["Tb"]
    g_ta, g_tb, g_t = meta["g_ta"], meta["g_tb"], meta["g_t"]
    win_tiles = meta["win_tiles"]
    SUM_TA, SUM_TB = sum(Ta), sum(Tb)
    SUM_T = SUM_TA + SUM_TB
    GT_MAX = max(g_t)
    offA = np.concatenate([[0], np.cumsum(g_ta)]).astype(int)
    offB = np.concatenate([[0], np.cumsum(g_tb)]).astype(int)
    offT = np.concatenate([[0], np.cumsum(g_t)]).astype(int)
    AF = mybir.ActivationFunctionType
    TT = mybir.AluOpType

    nc.gpsimd.load_library(mlp)

    import contextlib
    ctx = contextlib.ExitStack()
    with ctx:
        const = ctx.enter_context(tc.tile_pool(name="const", bufs=1))
        sb = ctx.enter_context(tc.tile_pool(name="sb", bufs=2))
        sb3 = ctx.enter_context(tc.tile_pool(name="sb3", bufs=3))
        ps = ctx.enter_context(tc.tile_pool(name="ps", bufs=2, space="PSUM"))

        # ---------- resident constants ----------
        ident = const.tile([P, P], bf16)
        make_identity(nc, ident[:])
        iota_r = const.tile([P, P], i8)      # row  iota: [p, f] = f
        nc.gpsimd.iota(iota_r[:], pattern=[[1, P]], base=0, channel_multiplier=0,
                       allow_small_or_imprecise_dtypes=True)
        iota_p = const.tile([P, 1], i8)      # partition iota: [p, 0] = p
        nc.gpsimd.iota(iota_p[:], pattern=[[1, 1]], base=0, channel_multiplier=1,
                       allow_small_or_imprecise_dtypes=True)

        xT_t = const.tile([P, NW, IN], bf16)
        nc.sync.dma_start(out=xT_t[:], in_=t["xT"][:].rearrange("p (w i) -> p w i", w=NW))
        w0_t = const.tile([IN, HID], bf16)
        nc.sync.dma_start(out=w0_t[:], in_=t["w0"][:])
        w0c_t = const.tile([IN, HID], bf16)
        nc.sync.dma_start(out=w0c_t[:], in_=t["w0c"][:])
        wc_t = const.tile([P, L - 1, 2, HID], bf16)
        nc.sync.dma_start(out=wc_t[:], in_=t["wc"][:].rearrange("l k p h -> p l k h"))
        wl_t = const.tile([P, 2, NCL], bf16)
        nc.sync.dma_start(out=wl_t[:], in_=t["wl"][:].rearrange("k p h -> p k h"))
        aab_t = const.tile([P, L, 2, HID], bf16)
        nc.sync.dma_start(out=aab_t[:], in_=t["aab"][:].rearrange("l s p h -> p l s h"))
        aal_t = const.tile([P, 2, NCL], bf16)
        nc.sync.dma_start(out=aal_t[:], in_=t["aal"][:].rearrange("s p h -> p s h"))
        b0b_t = const.tile([P, HID], f32)
        nc.sync.dma_start(out=b0b_t[:], in_=t["b0b"][:])
        b0cb_t = const.tile([P, HID], f32)
        nc.sync.dma_start(out=b0cb_t[:], in_=t["b0cb"][:])
        bcb_t = const.tile([P, L, HID], f32)
        nc.sync.dma_start(out=bcb_t[:], in_=t["bcb"][:].rearrange("l p h -> p l h"))
        blb_t = const.tile([P, NCL], f32)
        nc.sync.dma_start(out=blb_t[:], in_=t["blb"][:])
        idx_lo_t = const.tile([P, SUM_TA * 8], mybir.dt.int16)
        nc.sync.dma_start(out=idx_lo_t[:], in_=t["idx_lo"][:])
        idx_hi_t = const.tile([P, SUM_TB * 8], mybir.dt.int16)
        nc.sync.dma_start(out=idx_hi_t[:], in_=t["idx_hi"][:])
        dstc_t = const.tile([P, SUM_T], i8)
        nc.sync.dma_start(out=dstc_t[:], in_=t["dstc"][:])
        ad_loc = const.tile([P, NW, HEADS], bf16)

        out_d = t["out"]
        EMIT_CC = cfg.get("EMIT_CC", True)
        STG = cfg.get("EDGE_STAGE", 9)

        def transform(l, w, src_sb):
            """Build table row for layer l (0..L) from node-major activations
            src_sb [P, HID] bf16 (ignored for l == 0, which uses xT), write to
            cc_in[l], and fire the AG chunk when w closes it."""
            final = l == L
            HO = NCL if final else HID
            NH = 1 if final else HEADS
            tf = ps.tile([P, HID], f32, tag="tf")
            if l == 0:
                nc.tensor.matmul(tf[:, :HO], lhsT=xT_t[:, w, :], rhs=w0c_t[:],
                                 start=True, stop=True)
            else:
                hT_ps = ps.tile([P, 2, P], bf16, tag="hT")
                for kk in range(2):
                    nc.tensor.transpose(out=hT_ps[:, kk, :],
                                        in_=src_sb[:, kk * P:(kk + 1) * P],
                                        identity=ident[:])
                hT_sb = sb.tile([P, 2, P], bf16, tag="hTs")
                nc.scalar.copy(out=hT_sb[:], in_=hT_ps[:])
                w_t = wl_t if final else wc_t[:, l - 1, :, :]
                for kk in range(2):
                    nc.tensor.matmul(tf[:, :HO], lhsT=hT_sb[:, kk, :],
                                     rhs=w_t[:, kk, :HO],
                                     start=(kk == 0), stop=(kk == 1))
            RC = ROWF if final else ROW
            tbl = sb.tile([P, RC], bf16, tag="tbl")
            if l == 0:
                nc.vector.tensor_add(out=tbl[:, :HO], in0=tf[:, :HO], in1=b0cb_t[:])
            else:
                nc.scalar.copy(out=tbl[:, :HO], in_=tf[:, :HO])
            # alpha_src / alpha_dst from the table row
            aa = aal_t if final else aab_t[:, l, :, :]
            tmp = sb.tile([P, 2, HID], bf16, tag="atmp")
            nc.vector.tensor_tensor(
                out=tmp[:, :, :HO],
                in0=tbl[:, :HO].rearrange("p (o h) -> p o h", o=1).to_broadcast([P, 2, HO]),
                in1=aa[:, :, :HO], op=TT.mult)
            af = sb.tile([P, 2, HEADS], f32, tag="af")
            nc.vector.reduce_sum(
                af[:, :, :NH],
                tmp[:, :, :HO].rearrange("p o (h c) -> p o h c", h=NH),
                axis=mybir.AxisListType.X)
            nc.vector.tensor_copy(out=tbl[:, HO:HO + NH], in_=af[:, 0, :NH])
            nc.vector.tensor_copy(out=ad_loc[:, w, :NH], in_=af[:, 1, :NH])
            nc.sync.dma_start(out=t["cc_in"][l][w * P:(w + 1) * P, :HO + NH],
                              in_=tbl[:, :HO + NH])
            # fire AG chunk if w is its last window
            c = _chunk_of_window(cfg, w)
            if EMIT_CC and w == CH_W0[c + 1] - 1:
                w0_, w1_ = CH_W0[c], CH_W0[c + 1]
                nc.gpsimd.collective_compute(
                    "AllGather", TT.bypass,
                    replica_groups=[list(range(CORES))],
                    ins=[t["cc_in"][l].ap()[w0_ * P:w1_ * P, :].opt()],
                    outs=[t["tab"][l].ap()[CH_BASE[c]:CH_BASE[c + 1], :].opt()],
                )

        for _rep in range(rep):
            # ---------- embed + layer-0 table ----------
            for w in range(NW):
                tf = ps.tile([P, HID], f32, tag="tf")
                nc.tensor.matmul(tf[:], lhsT=xT_t[:, w, :], rhs=w0_t[:],
                                 start=True, stop=True)
                h0f = sb.tile([P, HID], f32, tag="hf")
                nc.vector.tensor_add(out=h0f[:], in0=tf[:], in1=b0b_t[:])
                h0r = sb.tile([P, HID], bf16, tag="hr")
                nc.scalar.copy(out=h0r[:], in_=h0f[:])
                nc.sync.dma_start(out=out_d[w * P:(w + 1) * P, 0:HID], in_=h0r[:])
                transform(0, w, None)

            # ---------- layers ----------
            for l in range(L + 1):
                final = l == L
                HO = NCL if final else HID
                NH = 1 if final else HEADS
                CH = HO // NH
                RC = ROWF if final else ROW
                col0 = HID * (l + 1)
                tab = t["tab"][l]
                tab_lo = tab.ap()[0:LO_ROWS]
                tab_hi = tab.ap()[LO_ROWS:]
                for gi, g in enumerate(GROUPS):
                    gta, gtb, gt = g_ta[gi], g_tb[gi], g_t[gi]
                    buf = sb.tile([P, GT_MAX, RC], bf16, tag="buf")
                    bufv = buf[:]
                    nc.gpsimd.dma_gather(
                        bufv[:, 0:gta, :], tab_lo,
                        idx_lo_t[:, offA[gi] * 8:(offA[gi] + gta) * 8],
                        gta * P, gta * P, RC, single_packet=False)
                    nc.gpsimd.dma_gather(
                        bufv[:, gta:gt, :], tab_hi,
                        idx_hi_t[:, offB[gi] * 8:(offB[gi] + gtb) * 8],
                        gtb * P, gtb * P, RC, single_packet=False)
                    if STG <= 1:
                        continue
                    dstT_t = sb.tile([P, GT_MAX * P], i8, tag="dstT")
                    nc.sync.dma_start(
                        out=dstT_t[:, :gt * P],
                        in_=t["dstT"][:, offT[gi] * P:(offT[gi] + gt) * P])
                    oh = sb.tile([P, GT_MAX, P], bf16, tag="oh")
                    nc.vector.tensor_tensor(
                        out=oh[:, :gt, :],
                        in0=dstc_t[:, offT[gi]:offT[gi] + gt]
                            .rearrange("p (t o) -> p t o", o=1).to_broadcast([P, gt, P]),
                        in1=iota_r[:].rearrange("p (o f) -> p o f", o=1)
                            .to_broadcast([P, gt, P]),
                        op=TT.is_equal)
                    ohT = sb.tile([P, GT_MAX, P], bf16, tag="ohT")
                    nc.vector.tensor_tensor(
                        out=ohT[:, :gt, :].rearrange("p t f -> p (t f)"),
                        in0=iota_p[:].to_broadcast([P, gt * P]),
                        in1=dstT_t[:, :gt * P],
                        op=TT.is_equal)
                    if STG <= 2:
                        continue
                    e_ps = ps.tile([P, GT_MAX * HEADS], f32, tag="eps")
                    for w in g:
                        for tt in win_tiles[w][0]:
                            nc.tensor.matmul(e_ps[:, tt * NH:(tt + 1) * NH],
                                             lhsT=ohT[:, tt, :],
                                             rhs=ad_loc[:, w, :NH],
                                             start=True, stop=True)
                    if STG <= 3:
                        continue
                    e_sb = sb.tile([P, GT_MAX, HEADS], f32, tag="esb")
                    nc.vector.tensor_tensor(
                        out=e_sb[:, :gt, :NH],
                        in0=bufv[:, :gt, HO:HO + NH],
                        in1=e_ps[:, :gt * NH].rearrange("p (t h) -> p t h", h=NH),
                        op=TT.add)
                    e2 = sb.tile([P, GT_MAX, HEADS], f32, tag="e2")
                    nc.vector.tensor_scalar_mul(e2[:, :gt, :NH], e_sb[:, :gt, :NH], 0.2)
                    nc.vector.tensor_tensor(out=e_sb[:, :gt, :NH],
                                            in0=e_sb[:, :gt, :NH],
                                            in1=e2[:, :gt, :NH], op=TT.max)
                    nc.scalar.activation(
                        bufv[:, :gt, HO:HO + NH],
                        e_sb[:, :gt, :NH], AF.Exp)
                    if STG <= 4:
                        continue
                    nc.vector.tensor_tensor(
                        out=bufv[:, :gt, :HO].rearrange("p t (h c) -> p t h c", h=NH),
                        in0=bufv[:, :gt, :HO].rearrange("p t (h c) -> p t h c", h=NH),
                        in1=bufv[:, :gt, HO:HO + NH].rearrange("p t (h o) -> p t h o", h=NH)
                            .to_broadcast([P, gt, NH, CH]),
                        op=TT.mult)
                    if STG <= 5:
                        continue
                    for w in g:
                        tiles = win_tiles[w][0]
                        o_ps = ps.tile([P, HID + HEADS], f32, tag="ops")
                        for j, tt in enumerate(tiles):
                            nc.tensor.matmul(o_ps[:, :HO + NH],
                                             lhsT=oh[:, tt, :],
                                             rhs=bufv[:, tt, :HO + NH],
                                             start=(j == 0), stop=(j == len(tiles) - 1))
                        if STG <= 6:
                            continue
                        den = sb.tile([P, HEADS], f32, tag="den")
                        nc.vector.tensor_scalar_add(den[:, :NH], o_ps[:, HO:HO + NH], 1e-16)
                        nc.vector.reciprocal(den[:, :NH], den[:, :NH])
                        hf = sb.tile([P, HID], f32, tag="hf")
                        nc.vector.tensor_tensor(
                            out=hf[:, :HO].rearrange("p (h c) -> p h c", h=NH),
                            in0=o_ps[:, :HO].rearrange("p (h c) -> p h c", h=NH),
                            in1=den[:, :NH].rearrange("p (h o) -> p h o", h=NH)
                                .to_broadcast([P, NH, CH]),
                            op=TT.mult)
                        bias = blb_t[:, :HO] if final else bcb_t[:, l, :HO]
                        nc.vector.tensor_add(out=hf[:, :HO], in0=hf[:, :HO], in1=bias)
                        hr = sb.tile([P, HID], bf16, tag="hr")
                        if final:
                            nc.scalar.copy(out=hr[:, :HO], in_=hf[:, :HO])
                        else:
                            nc.scalar.activation(hr[:, :HO], hf[:, :HO], AF.Relu)
                        nc.sync.dma_start(out=out_d[w * P:(w + 1) * P, col0:col0 + HO],
                                          in_=hr[:, :HO])
                        if not final:
                            transform(l + 1, w, hr)


# ------------------------------------------------------------------ driver


def _make_inmaps(inputs, meta, cfg):
    CORES, SH, NW, SHP = (cfg[k] for k in ("CORES", "SH", "NW", "SHP"))
    IN, HID, NCL, L = (cfg[k] for k in ("IN", "HID", "NC", "L"))

    x = np.asarray(inputs["x"])
    W0 = np.asarray(inputs["W0"]).astype(np.float32)
    Wc = np.asarray(inputs["Wc"]).astype(np.float32)
    Wl = np.asarray(inputs["Wl"]).astype(np.float32)
    a_src_c = np.asarray(inputs["a_src_c"]).reshape(L, HID)
    a_dst_c = np.asarray(inputs["a_dst_c"]).reshape(L, HID)
    a_src_l = np.asarray(inputs["a_src_l"]).reshape(NCL)
    a_dst_l = np.asarray(inputs["a_dst_l"]).reshape(NCL)
    b0 = np.asarray(inputs["b0"]).astype(np.float32)
    bc = np.asarray(inputs["bc"]).astype(np.float32)
    bl = np.asarray(inputs["bl"]).astype(np.float32)

    W0c = W0 @ Wc[0]                      # fused layer-0 table weight
    b0c = b0 @ Wc[0]

    def bcast(v, dt):
        return np.tile(v[None, :], (P, 1)).astype(dt)

    shared = dict(
        w0=W0.astype(BF), w0c=W0c.astype(BF),
        wc=Wc[1:].reshape(L - 1, 2, P, HID).astype(BF),
        wl=Wl.reshape(2, P, NCL).astype(BF),
        aab=np.stack([np.stack([bcast(a_src_c[l], BF), bcast(a_dst_c[l], BF)])
                      for l in range(L)]),
        aal=np.stack([bcast(a_src_l, BF), bcast(a_dst_l, BF)]),
        b0b=bcast(b0, np.float32), b0cb=bcast(b0c, np.float32),
        bcb=np.stack([bcast(bc[l], np.float32) for l in range(L)]),
        blb=bcast(bl, np.float32),
    )
    maps = []
    for k in range(CORES):
        xl = np.zeros((SHP, IN), np.float32)
        xl[:SH] = x[k * SH:(k + 1) * SH]
        xTl = np.ascontiguousarray(xl.reshape(NW, P, IN).transpose(2, 0, 1))
        maps.append(dict(shared,
                         xT=xTl.reshape(P, NW * IN).astype(BF),
                         idx_lo=meta["idx_lo"][k], idx_hi=meta["idx_hi"][k],
                         dstc=meta["dstc"][k], dstT=meta["dstT"][k]))
    return maps


_CACHE = {}


def _prep(inputs, cfg, rep=1):
    ck = ("meta", cfg["N"], cfg["E"])
    if ck not in _CACHE:
        _CACHE[ck] = _preprocess(np.asarray(inputs["edge_index"]), cfg)
    meta = _CACHE[ck]
    bk = ("nc", cfg["N"], cfg["E"], rep)
    if bk not in _CACHE:
        _CACHE[bk] = _build(meta, cfg, rep)
    mk = ("maps", cfg["N"], cfg["E"])
    if mk not in _CACHE:
        _CACHE[mk] = _make_inmaps(inputs, meta, cfg)
    return meta, _CACHE[bk], _CACHE[mk]


def _make_timed_callable(nc, in_maps, n_cores):
    import jax
    from jax.sharding import Mesh, PartitionSpec
    from jax.experimental.shard_map import shard_map
    import concourse.mybir as mybir_
    from concourse import bass2jax as b2j

    b2j.install_neuronx_cc_hook()
    partition_name = nc.partition_id_tensor.name if nc.partition_id_tensor else None
    in_names, out_names, out_avals, zero_outs = [], [], [], []
    for alloc in nc.m.functions[0].allocations:
        if not isinstance(alloc, mybir_.MemoryLocationSet):
            continue
        name = alloc.memorylocations[0].name
        if alloc.kind == "ExternalInput":
            if name != partition_name:
                in_names.append(name)
        elif alloc.kind == "ExternalOutput":
            shape = tuple(alloc.tensor_shape)
            dtype = mybir_.dt.np(alloc.dtype)
            out_names.append(name)
            out_avals.append(jax.core.ShapedArray(shape, dtype))
            zero_outs.append(np.zeros(shape, dtype))
    n_params = len(in_names)
    all_in = in_names + out_names + ([partition_name] if partition_name else [])

    def _body(*args):
        operands = list(args)
        if partition_name is not None:
            operands.append(b2j.partition_id_tensor())
        return tuple(b2j._bass_exec_p.bind(
            *operands, out_avals=tuple(out_avals), in_names=tuple(all_in),
            out_names=tuple(out_names), lowering_input_output_aliases=(),
            sim_require_finite=True, sim_require_nnan=True, nc=nc))

    devices = jax.devices()[:n_cores]
    mesh = Mesh(np.asarray(devices), ("core",))
    nin = n_params + len(out_names)
    sharded = jax.jit(shard_map(_body, mesh=mesh,
                                in_specs=(PartitionSpec("core"),) * nin,
                                out_specs=(PartitionSpec("core"),) * len(out_names),
                                check_rep=False), keep_unused=True)
    concat_in = [np.concatenate([np.asarray(in_maps[c][nm]) for c in range(n_cores)],
                                axis=0) for nm in in_names]
    concat_zeros = [np.zeros((n_cores * z.shape[0], *z.shape[1:]), z.dtype)
                    for z in zero_outs]
    sharding = jax.sharding.NamedSharding(mesh, PartitionSpec("core"))
    dev_args = [jax.device_put(a, sharding) for a in concat_in + concat_zeros]

    def call():
        outs = sharded(*dev_args)
        jax.block_until_ready(outs)
        return outs
    return call


def timed_run(inputs, reps=9, trials=12):
    """Median-slope timing: launch overhead is huge and noisy (tens of ms),
    so difference rep-1 and rep-N programs with medians over many trials."""
    import time as _t
    cfg = _derive(FULL_CFG)
    _, nc1, in_maps = _prep(inputs, cfg, rep=1)
    _, ncR, _ = _prep(inputs, cfg, rep=reps)
    f1 = _make_timed_callable(nc1, in_maps, cfg["CORES"])
    fR = _make_timed_callable(ncR, in_maps, cfg["CORES"])
    f1(); fR()
    t1s, tRs = [], []
    for _ in range(trials):
        t0 = _t.time(); f1(); t1s.append(_t.time() - t0)
        t0 = _t.time(); fR(); tRs.append(_t.time() - t0)
    m1, mR = np.median(t1s), np.median(tRs)
    print(f"[timing] rep1 med {m1*1e3:.2f} ms  rep{reps} med {mR*1e3:.2f} ms "
          f"(mins {min(t1s)*1e3:.2f}/{min(tRs)*1e3:.2f})")
    return (mR - m1) / (reps - 1) * 1e9


def _run(inputs, cfg, sim_check=False):
    meta, nc, in_maps = _prep(inputs, cfg)
    SH = cfg["SH"]
    if sim_check:
        from concourse.bass_interp import MultiCoreSim
        sim = MultiCoreSim(nc, num_cores=cfg["CORES"], require_finite=False,
                           require_nnan=False)
        for k, core in sim.cores.items():
            for name, arr in in_maps[k].items():
                core.tensor(name)[:] = arr
        sim.simulate(check_with_hw=False)
        outs = [np.array(sim.cores[k].tensor("out")) for k in range(cfg["CORES"])]
    else:
        res = run_bass_kernel_spmd(nc, in_maps,
                                   core_ids=list(range(cfg["CORES"])))
        outs = [res.results[k]["out"] for k in range(cfg["CORES"])]
    return np.concatenate([o[:SH] for o in outs], axis=0).astype(np.float32)


def kernel(**inputs) -> np.ndarray:
    cfg = _derive(FULL_CFG)
    return _run(inputs, cfg)
